# revision 58
# baseline (speedup 1.0000x reference)
"""Trainium2 Bass kernel for the BAN (bilinear attention network) problem.

Math (per batch b, eval mode):
    hq = emb[he_ques] @ Wq + bq                  [NQ, H]
    hk = emb[he_kg]   @ Wk + bk                  [NK, H]
    logits[g,q,k] = sum_d hq[q,d] Watt[d,g] hk[k,d]   (+ batt[g], cancels in
                                                       the joint softmax)
    att = softmax over flattened (q,k) per (b,g)
    pooled[g,d] = sum_{q,k} hq[q,d] att[g,q,k] hk[k,d]
    out = pooled.flat @ Wout + bout;  sim = out @ glove.T;  log_softmax(sim)

Distribution: pure data parallel over batch, 8 samples per core on 8 cores.
All weights replicated. No collectives.

v7 design notes (bf16 datapath of v5 + restructured schedule):
  - fp8/DoubleRow was tried (v6) and REVERTED: DR halves matmul row time
    in theory but every fp8 elementwise op (exp, copies, broadcasts,
    reduces) runs 1.5-2x slower on Scalar/DVE/GpSimd, and those engines
    are the real bottleneck; net was a big regression.
  - Startup: gathers ordered k0,k1,q0,q1,k2,k3; XBAR transposes split
    across BOTH hwdge queues (k0,k1,q0 on Sync; q1,k2,k3 on Scalar).
    Phase C (hq projection) runs before D2-pair0 on the PE; only a
    short warmup burst remains for the HAM clock ramp.
  - hqw produced per (m, sample-pair): pairs 0,1 on DVE right after
    phase C (ready before D5 sample 0); pairs 2,3 on GpSimd emitted
    early in the loop. Removes the v5 phase-C stall region entirely.
  - lg PSUM double-buffered (D5 of b+1 no longer waits D6 exp of b);
    mm pool at 2 bufs to pay for it.
  - poT scale on GpSimd (SBUF-only op; DVE is the loop's 2nd engine).
  - Tail: sim-phase Exp uses accum_out (Z partial sums ride the Scalar
    op; no DVE reduces); final (sim - lnZ) split DVE/Scalar with
    staggered output DMAs.
"""

import sys

if "/opt/trn_rl_repo" not in sys.path:
    sys.path.insert(0, "/opt/trn_rl_repo")

import numpy as np

import concourse.bass as bass
import concourse.mybir as mybir
import concourse.tile as tile
from concourse import bacc
from concourse.bass_utils import run_bass_kernel_spmd

F32 = mybir.dt.float32
BF16 = mybir.dt.bfloat16
I32 = mybir.dt.int32
AX = mybir.AxisListType
OP = mybir.AluOpType
AF = mybir.ActivationFunctionType

N_CORES = 8
VOCAB = 20000
E = 300          # word embedding size
EA = 384         # padded: col 300 = ones (bias trick), 301.. = zeros
H = 1024         # hidden
G = 8            # heads
N_OUT = 300
N_ANS = 4000
B, NQ, NK = 64, 32, 256
BL = B // N_CORES            # 8 samples per core
TQ = BL * NQ                 # 256 q tokens per core
TK = BL * NK                 # 2048 k tokens per core
TQ_TILES = TQ // 128         # 2
TK_TILES = TK // 128         # 16
DT = H // 128                # 8 d-tiles
N_CHUNKS = (128, 128, N_OUT - 256)   # (128, 128, 44) rows of the 300-dim
NA_CH = 8                    # sim computed in 8 chunks of 500
NA_W = N_ANS // NA_CH        # 500
NWOUT = G * DT               # 64 Wout k-tiles

DEBUG_TAPS = False


def build_kernel(num_devices=N_CORES):
    nc = bacc.Bacc("TRN2", target_bir_lowering=False, debug=False,
                   num_devices=num_devices)

    # ---- DRAM I/O ----
    emb_d = nc.dram_tensor("emb", [VOCAB, EA], BF16, kind="ExternalInput").ap()
    idxq_d = nc.dram_tensor("idx_q", [128, TQ_TILES], I32, kind="ExternalInput").ap()
    idxk_d = nc.dram_tensor("idx_k", [128, TK_TILES], I32, kind="ExternalInput").ap()
    wq_d = nc.dram_tensor("wq", [EA, H], BF16, kind="ExternalInput").ap()
    wk_d = nc.dram_tensor("wk", [EA, H], BF16, kind="ExternalInput").ap()
    watt_d = nc.dram_tensor("watt", [128, DT, G], BF16, kind="ExternalInput").ap()
    wout_d = nc.dram_tensor("wout", [G * H, N_OUT], BF16, kind="ExternalInput").ap()
    bout_d = nc.dram_tensor("bout", [BL, N_OUT], F32, kind="ExternalInput").ap()
    glovet_d = nc.dram_tensor("glovet", [N_OUT, N_ANS], BF16,
                              kind="ExternalInput").ap()
    out_d = nc.dram_tensor("out", [BL, N_ANS], F32, kind="ExternalOutput").ap()
    warm_d = nc.dram_tensor("warm", [1, 128], F32, kind="ExternalOutput").ap()
    if DEBUG_TAPS:
        dbg_hqt_d = nc.dram_tensor("dbg_hqt", [128, DT, TQ], BF16,
                                   kind="ExternalOutput").ap()
        dbg_hqw_d = nc.dram_tensor("dbg_hqw", [128, DT, 2, G, NQ], BF16,
                                   kind="ExternalOutput").ap()
        dbg_hkt_d = nc.dram_tensor("dbg_hkt", [128, DT, 512], BF16,
                                   kind="ExternalOutput").ap()
        dbg_hk_d = nc.dram_tensor("dbg_hk", [128, 4, H], BF16,
                                  kind="ExternalOutput").ap()
        dbg_et_d = nc.dram_tensor("dbg_et", [128, 2, G * NQ], BF16,
                                  kind="ExternalOutput").ap()
        dbg_vr_d = nc.dram_tensor("dbg_vr", [128, DT, G], F32,
                                  kind="ExternalOutput").ap()
        dbg_pot_d = nc.dram_tensor("dbg_pot", [128, DT, G, BL], BF16,
                                   kind="ExternalOutput").ap()

    with tile.TileContext(nc) as tc:
        import contextlib

        with contextlib.ExitStack() as ctx:
            consts = ctx.enter_context(tc.tile_pool(name="consts", bufs=1))
            actx = contextlib.ExitStack()
            hqw_pool = actx.enter_context(tc.tile_pool(name="hqwp", bufs=1))
            xrow_p = actx.enter_context(tc.tile_pool(name="xrow", bufs=6))
            xkt_p = actx.enter_context(tc.tile_pool(name="xkt", bufs=2))
            hkt_p = actx.enter_context(tc.tile_pool(name="hkt", bufs=2))
            hk_p = actx.enter_context(tc.tile_pool(name="hk", bufs=2))
            et_p = actx.enter_context(tc.tile_pool(name="et", bufs=2))
            v_p = actx.enter_context(tc.tile_pool(name="v", bufs=3))
            vr_p = actx.enter_context(tc.tile_pool(name="vr", bufs=3))
            zz_p = actx.enter_context(tc.tile_pool(name="zz", bufs=2))
            zn_p = actx.enter_context(tc.tile_pool(name="zn", bufs=3))
            mm_p = actx.enter_context(tc.tile_pool(name="mm", bufs=2, space="PSUM"))
            tp_p = actx.enter_context(tc.tile_pool(name="tp", bufs=2, space="PSUM"))
            lg_p = actx.enter_context(tc.tile_pool(name="lg", bufs=2, space="PSUM"))
            up_p = actx.enter_context(tc.tile_pool(name="up", bufs=2, space="PSUM"))

            # ---- on-chip constants (no DMA round-trip) ----
            ident = consts.tile([128, 128], BF16, tag="ident")
            nc.gpsimd.memset(ident[:], 1.0)
            nc.gpsimd.affine_select(
                out=ident[:], in_=ident[:], pattern=[[-1, 128]], base=0,
                channel_multiplier=1, compare_op=OP.is_equal, fill=0.0,
            )
            ones_sb = consts.tile([128, 1], F32, tag="ones")
            nc.gpsimd.memset(ones_sb[:], 1.0)
            ones_row = consts.tile([1, 128], F32, tag="ones_row")
            nc.gpsimd.memset(ones_row[:], 1.0)
            wz = consts.tile([128, 512], BF16, tag="wz")
            nc.vector.memset(wz[:], 0.0)

            # ---- critical input DMAs (sync queue, tiny idx first) ----
            idxk_sb = consts.tile([128, TK_TILES], I32, tag="idxk")
            nc.sync.dma_start(idxk_sb[:], idxk_d)
            idxq_sb = consts.tile([128, TQ_TILES], I32, tag="idxq")
            nc.sync.dma_start(idxq_sb[:], idxq_d)
            wq_sb = consts.tile([128, 3, H], BF16, tag="wq")
            nc.sync.dma_start(
                wq_sb[:], wq_d.rearrange("(c p) h -> p c h", p=128))
            wk_sb = consts.tile([128, 3, H], BF16, tag="wk")
            watt_sb = consts.tile([128, DT, G], BF16, tag="watt")

            # ---- gathers on gpsimd: k0,k1 / q0,q1 / k2,k3 ----
            xrow_tiles = {}

            def gather(idx_sb, col):
                xrow = xrow_p.tile([128, EA], BF16, tag="xrow")
                nc.gpsimd.indirect_dma_start(
                    out=xrow[:],
                    out_offset=None,
                    in_=emb_d,
                    in_offset=bass.IndirectOffsetOnAxis(
                        ap=idx_sb[:, col : col + 1], axis=0
                    ),
                )
                return xrow

            def transpose_x(xrow, dst, dst_col):
                """Transpose one gathered X tile [k, e] -> [e, k] on the PE
                (3 128x128 blocks into PSUM) + one Scalar copy. The XBAR
                DMA_TRANSPOSE path is avoided entirely: hwdge transposes
                serialize ~1.3us each AND acquire conservative waits on ALL
                in-flight DMAs, which repeatedly stalled the critical path
                for tens of us."""
                pt = tp_p.tile([128, 512], BF16, tag="mmT")
                for c in range(3):
                    nc.tensor.transpose(
                        pt[:, c * 128 : (c + 1) * 128],
                        xrow[:, c * 128 : (c + 1) * 128],
                        ident[:],
                    )
                nc.scalar.activation(
                    out=dst[:, :, dst_col * 128 : (dst_col + 1) * 128],
                    in_=pt[:, :384].rearrange("p (c f) -> p c f", c=3),
                    func=AF.Copy)

            # PE warmup: short bf16 burst to push the HAM clock ramp while
            # the gathers land.
            wps = up_p.tile([128, 512], F32, tag="up")
            for _ in range(10):
                nc.tensor.matmul(wps[:], lhsT=ident[:], rhs=wz[:],
                                 start=True, stop=True)
            warm_sb = consts.tile([1, 128], F32, tag="warm")
            nc.vector.tensor_copy(warm_sb[:], wps[:1, :128])
            nc.sync.dma_start(warm_d, warm_sb[:])

            xqT = consts.tile([128, 3, TQ], BF16, tag="xqT")
            xkT0 = xkt_p.tile([128, 3, 512], BF16, tag="xkT")
            for t in range(TQ_TILES):
                xrow_tiles[("q", t)] = gather(idxq_sb, t)
            for t in range(4):
                xrow_tiles[("k", t)] = gather(idxk_sb, t)
            # wk/watt DMAs emitted AFTER the gathers: the PE's first x
            # transpose conservatively waits all DMAs emitted before it
            nc.sync.dma_start(
                wk_sb[:], wk_d.rearrange("(c p) h -> p c h", p=128))
            nc.sync.dma_start(watt_sb[:], watt_d)
            transpose_x(xrow_tiles.pop(("q", 0)), xqT, 0)
            transpose_x(xrow_tiles.pop(("q", 1)), xqT, 1)

            # ---- phase C: hqT (PE before D2 pair0; needs only q gathers) --
            # hqw: ONE TILE PER SAMPLE-PAIR, layout [d, m, 2, g, q]. The
            # single-tile variant with [:, m, 2bp:2bp+2] write slices raced
            # its readers (subtile dep tracking missed the b-sliced 5-D
            # writes); whole-[:, m] writes per tile are tracked correctly.
            hqT = consts.tile([128, DT, TQ], BF16, tag="hqT")
            hqw_t = [hqw_pool.tile([128, DT, 2, G, NQ], BF16, tag=f"hqw{bp}",
                                   name=f"hqw{bp}")
                     for bp in range(4)]
            for m in range(DT):
                ps = mm_p.tile([128, 512], F32, tag="mm")
                for c in range(3):
                    nc.tensor.matmul(
                        ps[:, :TQ],
                        lhsT=wq_sb[:, c, m * 128 : (m + 1) * 128],
                        rhs=xqT[:, c, :],
                        start=(c == 0),
                        stop=(c == 2),
                    )
                nc.vector.tensor_copy(out=hqT[:, m, :], in_=ps[:, :TQ])
            # k-tile transposes on the PE right after phase C (each waits
            # only its own gather's completion)
            for t in range(4):
                transpose_x(xrow_tiles.pop(("k", t)), xkT0, t)

            # ALL hqw ops stay on DVE: offloading them to GpSimd was tried
            # twice and reverted -- concurrent GpSimd+DVE broadcast ops
            # lockstep each other at ~2x duration (shared SBUF path), even
            # with fully private input tensors.
            def emit_hqw(eng, m, bp):
                """hqw for d-tile m, sample pair bp (2 samples)."""
                with nc.allow_low_precision(reason="bf16 hqw"):
                    eng.tensor_tensor(
                        out=hqw_t[bp][:, m],
                        in0=hqT[:, m, :].rearrange("p (b q) -> p b q", b=BL)[
                            :, 2 * bp : 2 * bp + 2, None, :
                        ].to_broadcast([128, 2, G, NQ]),
                        in1=watt_sb[:, m, None, :, None].to_broadcast(
                            [128, 2, G, NQ]),
                        op=OP.mult,
                    )

            # pair 0 on DVE right after the hqT copies: ready before D5
            # sample 0 consumes it (pair 1 emitted after pair0's hkT copies)
            for m in range(DT):
                emit_hqw(nc.vector, m, 0)

            if DEBUG_TAPS:
                nc.sync.dma_start(dbg_hqt_d, hqT[:])
                dbg_hqw = consts.tile([128, DT, 2, G, NQ], BF16, tag="dbg_hqw")
                nc.vector.tensor_copy(dbg_hqw[:], hqw_t[0][:])
                nc.sync.dma_start(dbg_hqw_d, dbg_hqw[:])

            def project_k_pair(xkT):
                """hkT [d, k-pair] by matmul (so D5 never waits on a DMA
                transpose), then token-major hk via PE transposes, lagged
                one d-tile so each hkT copy has landed before its
                transpose reads it."""
                hkT = hkt_p.tile([128, DT, 512], BF16, tag="hkT")
                hk = hk_p.tile([128, 4, H], BF16, tag="hk")

                def emit_d2(m):
                    ps = mm_p.tile([128, 512], F32, tag="mm")
                    for c in range(3):
                        nc.tensor.matmul(
                            ps[:],
                            lhsT=wk_sb[:, c, m * 128 : (m + 1) * 128],
                            rhs=xkT[:, c, :],
                            start=(c == 0),
                            stop=(c == 2),
                        )
                    if m % 2 == 0:
                        nc.scalar.activation(out=hkT[:, m, :], in_=ps[:],
                                             func=AF.Copy)
                    else:
                        nc.vector.tensor_copy(out=hkT[:, m, :], in_=ps[:])

                def emit_transpose(m):
                    pt = tp_p.tile([128, 512], BF16, tag="mmT")
                    for t in range(4):
                        nc.tensor.transpose(
                            pt[:, t * 128 : (t + 1) * 128],
                            hkT[:, m, t * 128 : (t + 1) * 128],
                            ident[:],
                        )
                    nc.scalar.activation(
                        out=hk[:, :, m * 128 : (m + 1) * 128],
                        in_=pt[:].rearrange("p (t f) -> p t f", t=4),
                        func=AF.Copy)

                emit_d2(0)
                for m in range(1, DT):
                    emit_d2(m)
                    emit_transpose(m - 1)
                emit_transpose(DT - 1)
                return hk, hkT

            # ---- D2 pair 0 ----
            hk_cur, hkT_cur = project_k_pair(xkT0)
            # pair 1's hqw after pair0's hkT copies on the DVE queue, so
            # D5 sample 0 isn't gated by these (deadline: sample 2)
            for m in range(DT):
                emit_hqw(nc.vector, m, 1)

            if DEBUG_TAPS:
                dbg_hkt = consts.tile([128, DT, 512], BF16, tag="dbg_hkt")
                nc.vector.tensor_copy(dbg_hkt[:], hkT_cur[:])
                nc.sync.dma_start(dbg_hkt_d, dbg_hkt[:])
                dbg_hk = consts.tile([128, 4, H], BF16, tag="dbg_hk")
                nc.vector.tensor_copy(dbg_hk[:], hk_cur[:])
                nc.sync.dma_start(dbg_hk_d, dbg_hk[:])

            # deferred weight streams, chunked so no single transfer
            # monopolizes the DMA path or skews semaphore completion order
            wout_sb = consts.tile([128, NWOUT, N_OUT], BF16, tag="wout")
            glove_sb = consts.tile([128, 3, N_ANS], BF16, tag="glove")
            bout_sb = consts.tile([BL, N_OUT], F32, tag="bout")

            def emit_weight_chunk(step):
                if step < 8:        # wout: 8 chunks of 8 k-tiles
                    lo = step * 8
                    nc.sync.dma_start(
                        wout_sb[:, lo : lo + 8, :],
                        wout_d[lo * 128 : (lo + 8) * 128].rearrange(
                            "(t p) n -> p t n", p=128))
                elif step < 10:     # glove rows 0..255 in 2 chunks
                    c = step - 8
                    nc.sync.dma_start(
                        glove_sb[:, c, :],
                        glovet_d[c * 128 : (c + 1) * 128])
                elif step == 10:    # glove rows 256..299
                    nc.sync.dma_start(glove_sb[: N_OUT - 256, 2, :],
                                      glovet_d[2 * 128 : N_OUT])
                elif step == 11:
                    nc.sync.dma_start(bout_sb[:], bout_d)

            poT = consts.tile([128, DT, G, BL], BF16, tag="poT")
            # unscaled pooled accumulator + per-sample 1/Z values: the
            # pooled scale is applied ONCE after the loop. The per-sample
            # recip->partition_broadcast->scale chain (DVE->GpSimd->DVE)
            # paid ~2.3us of semaphore latency per hop and head-of-line
            # blocked the in-order DVE queue, stalling the next sample.
            vrT = consts.tile([128, DT, G, BL], F32, tag="vrT")
            zall = consts.tile([1, G, BL], F32, tag="zall")
            wstep = 0

            # ---- phase D: attention, two samples per pair ----
            for p in range(BL // 2):
                hk, hkT = hk_cur, hkT_cur
                xkT_next = None
                if p < 3:
                    xkT_next = xkt_p.tile([128, 3, 512], BF16, tag="xkT")
                    for t in range(4):
                        xrow_tiles[("k", t)] = gather(idxk_sb, (p + 1) * 4 + t)


                for bi in range(2):
                    b = p * 2 + bi

                    # D5: logits.T [k, (g,q)] in PSUM: [128, 2, 256]
                    ps_l = lg_p.tile([128, 2, 256], F32, tag="lg")
                    for kt in range(2):
                        for c in range(DT):
                            nc.tensor.matmul(
                                ps_l[:, kt, :],
                                lhsT=hkT[
                                    :, c,
                                    bi * 256 + kt * 128 : bi * 256 + (kt + 1) * 128,
                                ],
                                rhs=hqw_t[b // 2][:, c, b % 2],
                                start=(c == 0),
                                stop=(c == DT - 1),
                            )

                    # D6: E = exp(logits) bf16 (one op), zz sums (one op)
                    et = et_p.tile([128, 2, G * NQ], BF16, tag="et")
                    zz = zz_p.tile([128, 2, G], F32, tag="zz")
                    nc.scalar.activation(
                        out=et[:], in_=ps_l[:], func=AF.Exp)
                    nc.vector.tensor_reduce(
                        out=zz[:],
                        in_=et[:].rearrange("p t (g q) -> p t g q", g=G),
                        axis=AX.X,
                        op=OP.add,
                    )

                    def emit_d7():
                        # D7: Z_g over k-partitions, 1/Z stashed for the
                        # batched scale. Emitted late (post-D8) so the PE
                        # never waits on zz -- except for the LAST sample,
                        # where it runs early so recip lands before the
                        # final pooled scale that gates phase F.
                        ps_z = mm_p.tile([128, 512], F32, tag="mm")
                        for kt in range(2):
                            nc.tensor.matmul(
                                ps_z[:1, :G],
                                lhsT=ones_sb[:],
                                rhs=zz[:, kt, :],
                                start=(kt == 0),
                                stop=(kt == 1),
                            )
                        nc.vector.reciprocal(zall[:1, :, b], ps_z[:1, :G])

                    if b == BL - 1:
                        emit_d7()

                    # D8: u = hk.T @ E per 2 d-tiles; v = u * hq; vr = sum_q
                    for mp in range(4):
                        ps_u = up_p.tile([128, 512], F32, tag="up")
                        for mi in range(2):
                            m = mp * 2 + mi
                            for kt in range(2):
                                nc.tensor.matmul(
                                    ps_u[:, mi * 256 : (mi + 1) * 256],
                                    lhsT=hk[:, bi * 2 + kt, m * 128 : (m + 1) * 128],
                                    rhs=et[:, kt, :],
                                    start=(kt == 0),
                                    stop=(kt == 1),
                                )
                        v = v_p.tile([128, 2, G, NQ], BF16, tag="v")
                        with nc.allow_low_precision(reason="bf16 v"):
                            nc.vector.tensor_tensor(
                                out=v[:],
                                in0=ps_u[:].rearrange(
                                    "p (m g q) -> p m g q", m=2, g=G),
                                in1=hqT[
                                    :, mp * 2 : mp * 2 + 2, None,
                                    b * NQ : (b + 1) * NQ
                                ].to_broadcast([128, 2, G, NQ]),
                                op=OP.mult,
                            )
                        nc.vector.tensor_reduce(
                            out=vrT[:, mp * 2 : mp * 2 + 2, :, b], in_=v[:],
                            axis=AX.X, op=OP.add,
                        )

                    if DEBUG_TAPS and b == 0:
                        dbg_et = consts.tile([128, 2, G * NQ], BF16,
                                             tag="dbg_et")
                        nc.vector.tensor_copy(dbg_et[:], et[:])
                        nc.sync.dma_start(dbg_et_d, dbg_et[:])
                        nc.sync.dma_start(dbg_vr_d, vrT[:, :, :, 0])

                    if b != BL - 1:
                        emit_d7()
                    # hqw for pairs 2,3 in 4-op slices woven into the DVE
                    # queue across samples 0-3 (fine grain so the up-pool
                    # rotation never backs up the PE)
                    if b < 4:
                        for mi in range(4):
                            emit_hqw(nc.vector, (b % 2) * 4 + mi, 2 + b // 2)

                    if bi == 0:
                        if p < 3:
                            # next pair's X transposes mid-pair (PE + Scalar
                            # copies; each waits only its own gather)
                            for t in range(4):
                                transpose_x(xrow_tiles.pop(("k", t)),
                                            xkT_next, t)
                        emit_weight_chunk(wstep); wstep += 1
                        emit_weight_chunk(wstep); wstep += 1

                emit_weight_chunk(wstep); wstep += 1
                if p < 3:
                    hk_cur, hkT_cur = project_k_pair(xkT_next)

            # batched pooled scale: one PE broadcast of 1/Z to all
            # partitions, one DVE multiply for all 8 samples
            zb_ps = mm_p.tile([128, 512], F32, tag="mm")
            nc.tensor.matmul(
                zb_ps[:, : G * BL],
                lhsT=ones_row[:],
                rhs=zall[:1].rearrange("o g b -> o (g b)"),
                start=True, stop=True,
            )
            with nc.allow_low_precision(reason="bf16 pooled"):
                nc.vector.tensor_tensor(
                    out=poT[:],
                    in0=vrT[:],
                    in1=zb_ps[:, : G * BL].rearrange(
                        "p (g b) -> p g b", g=G)[:, None, :, :]
                    .to_broadcast([128, DT, G, BL]),
                    op=OP.mult,
                )

            if DEBUG_TAPS:
                nc.sync.dma_start(dbg_pot_d, poT[:])

            # attention pools (incl. all PSUM) are dead now
            actx.close()
            fctx = contextlib.ExitStack()
            fo_p = fctx.enter_context(tc.tile_pool(name="fo", bufs=2, space="PSUM"))

            # ---- phase F: out [8, 300] = pooled_flat @ Wout + bout ----
            ps_o = fo_p.tile([128, 512], F32, tag="fo")
            for g in range(G):
                for m in range(DT):
                    t = g * DT + m
                    nc.tensor.matmul(
                        ps_o[:BL, :N_OUT],
                        lhsT=poT[:, m, g, :],
                        rhs=wout_sb[:, t, :],
                        start=(t == 0),
                        stop=(t == NWOUT - 1),
                    )
            out_sb = consts.tile([BL, N_OUT], BF16, tag="out_sb")
            with nc.allow_low_precision(reason="bf16 out"):
                nc.vector.tensor_tensor(
                    out=out_sb[:], in0=ps_o[:BL, :N_OUT], in1=bout_sb[:], op=OP.add
                )

            # ---- phase G: sim + log_softmax (no max shift; sim is O(+-5)) --
            outT = consts.tile([128, 3, BL], BF16, tag="outT")
            for c, rows in enumerate(N_CHUNKS):
                psT = fo_p.tile([128, 128], BF16, tag="foT")
                nc.tensor.transpose(
                    psT[:rows, :BL],
                    out_sb[:, c * 128 : c * 128 + rows],
                    ident[:BL, :BL],
                )
                nc.scalar.activation(out=outT[:rows, c, :], in_=psT[:rows, :BL],
                                     func=AF.Copy)

            zs8 = consts.tile([BL, NA_CH], F32, tag="zs8")
            zs = consts.tile([BL, 1], F32, tag="zs")
            zsi = consts.tile([BL, 1], F32, tag="zsi")
            nlnz = consts.tile([BL, 1], F32, tag="nlnz")
            final_sb = consts.tile([BL, N_ANS], F32, tag="final")
            simsb = consts.tile([BL, N_ANS], BF16, tag="simsb")

            # sim chunks on 4 rotating PSUM banks alongside fo's 2: no
            # fctx.close() barrier (its PE DRAIN burned ~7us mid-tail).
            # Each chunk is staged to SBUF bf16 so its bank recycles; the
            # final subtract reads the staged copy.
            esc_p = fctx.enter_context(tc.tile_pool(name="esc", bufs=2))
            sim_p = fctx.enter_context(tc.tile_pool(name="simp", bufs=4,
                                                    space="PSUM"))
            for a in range(NA_CH):
                span = slice(a * NA_W, (a + 1) * NA_W)
                ps_s = sim_p.tile([128, NA_W], F32, tag="simp")
                for c, rows in enumerate(N_CHUNKS):
                    nc.tensor.matmul(
                        ps_s[:BL, :],
                        lhsT=outT[:rows, c, :],
                        rhs=glove_sb[:rows, c, a * NA_W : (a + 1) * NA_W],
                        start=(c == 0),
                        stop=(c == 2),
                    )
                # Exp + DVE reduce (accum_out was tried: the per-op
                # ACTIVATION_READ_ACCUMULATOR + semaphore round-trip
                # serialized the Scalar chain at 1.43us/chunk)
                esc = esc_p.tile([BL, NA_W], BF16, tag="esc")
                nc.scalar.activation(out=esc[:], in_=ps_s[:BL, :], func=AF.Exp)
                nc.vector.tensor_reduce(
                    out=zs8[:, a : a + 1], in_=esc[:], axis=AX.X, op=OP.add
                )
                with nc.allow_low_precision(reason="bf16 sim stage"):
                    if a % 2 == 0:
                        nc.vector.tensor_copy(out=simsb[:, span],
                                              in_=ps_s[:BL, :])
                    else:
                        nc.scalar.activation(out=simsb[:, span],
                                             in_=ps_s[:BL, :], func=AF.Copy)

            nc.vector.tensor_reduce(out=zs[:], in_=zs8[:], axis=AX.X, op=OP.add)
            nc.vector.reciprocal(zsi[:], zs[:])
            nc.scalar.activation(out=nlnz[:], in_=zsi[:], func=AF.Ln)
            # final = sim - lnZ, chunks split DVE / Act / GpSimd (simsb is
            # SBUF so GpSimd may join), staggered DMAs
            for a in range(NA_CH):
                span = slice(a * NA_W, (a + 1) * NA_W)
                if a % 3 == 0:
                    nc.vector.tensor_scalar(
                        out=final_sb[:, span], in0=simsb[:, span],
                        scalar1=nlnz[:], scalar2=None,
                        op0=OP.add,
                    )
                elif a % 3 == 1:
                    nc.scalar.activation(
                        out=final_sb[:, span], in_=simsb[:, span],
                        func=AF.Identity, bias=nlnz[:],
                    )
                else:
                    nc.gpsimd.tensor_scalar(
                        out=final_sb[:, span], in0=simsb[:, span],
                        scalar1=nlnz[:], scalar2=None,
                        op0=OP.add,
                    )
                if a == 2:
                    nc.sync.dma_start(out_d[:, : 3 * NA_W],
                                      final_sb[:, : 3 * NA_W])
                if a == 5:
                    nc.sync.dma_start(out_d[:, 3 * NA_W : 6 * NA_W],
                                      final_sb[:, 3 * NA_W : 6 * NA_W])
            nc.sync.dma_start(out_d[:, 6 * NA_W :], final_sb[:, 6 * NA_W :])
            fctx.close()

    nc.compile()
    return nc


_NC = None


def _get_nc():
    global _NC
    if _NC is None:
        _NC = build_kernel()
    return _NC


def make_in_maps(inputs):
    import ml_dtypes

    bf = ml_dtypes.bfloat16
    he_q = np.asarray(inputs["he_ques"]).astype(np.int32)   # [64, 32]
    he_k = np.asarray(inputs["he_kg"]).astype(np.int32)     # [64, 256]
    emb0 = np.asarray(inputs["emb"], dtype=np.float32)
    emb = np.zeros((VOCAB, EA), dtype=bf)
    emb[:, :E] = emb0.astype(bf)
    emb[:, E] = np.ones((), dtype=bf)                       # bias column
    wq = np.zeros((EA, H), dtype=bf)
    wq[:E] = np.asarray(inputs["Wq"], np.float32).astype(bf)
    wq[E] = np.asarray(inputs["bq"], np.float32).astype(bf)
    wk = np.zeros((EA, H), dtype=bf)
    wk[:E] = np.asarray(inputs["Wk"], np.float32).astype(bf)
    wk[E] = np.asarray(inputs["bk"], np.float32).astype(bf)
    watt = np.ascontiguousarray(
        np.asarray(inputs["Watt"], np.float32).reshape(DT, 128, G)
        .transpose(1, 0, 2)).astype(bf)                     # [128, DT, G]
    wout = np.ascontiguousarray(
        np.asarray(inputs["Wout"], np.float32)).astype(bf)
    bout = np.ascontiguousarray(
        np.broadcast_to(np.asarray(inputs["bout"], np.float32), (BL, N_OUT)))
    glovet = np.ascontiguousarray(
        np.asarray(inputs["glove_cands"], np.float32).T).astype(bf)  # [300,4000]

    in_maps = []
    for i in range(N_CORES):
        iq = he_q[i * BL : (i + 1) * BL].reshape(-1)        # [256]
        ik = he_k[i * BL : (i + 1) * BL].reshape(-1)        # [2048]
        in_maps.append({
            "emb": emb,
            "idx_q": np.ascontiguousarray(iq.reshape(TQ_TILES, 128).T),
            "idx_k": np.ascontiguousarray(ik.reshape(TK_TILES, 128).T),
            "wq": wq,
            "wk": wk,
            "watt": watt,
            "wout": wout,
            "bout": bout,
            "glovet": glovet,
        })
    return in_maps


def kernel(**inputs) -> np.ndarray:
    nc = _get_nc()
    in_maps = make_in_maps(inputs)
    res = run_bass_kernel_spmd(nc, in_maps, list(range(N_CORES)))
    return np.concatenate(
        [np.asarray(res.results[i]["out"], np.float32) for i in range(N_CORES)],
        axis=0,
    )


# revision 59
# speedup vs baseline: 1.1058x; 1.1058x over previous
"""Trainium2 Bass kernel for the BAN (bilinear attention network) problem.

Math (per batch b, eval mode):
    hq = emb[he_ques] @ Wq + bq                  [NQ, H]
    hk = emb[he_kg]   @ Wk + bk                  [NK, H]
    logits[g,q,k] = sum_d hq[q,d] Watt[d,g] hk[k,d]   (+ batt[g], cancels in
                                                       the joint softmax)
    att = softmax over flattened (q,k) per (b,g)
    pooled[g,d] = sum_{q,k} hq[q,d] att[g,q,k] hk[k,d]
    out = pooled.flat @ Wout + bout;  sim = out @ glove.T;  log_softmax(sim)

Distribution: pure data parallel over batch, 8 samples per core on 8 cores.
All weights replicated. No collectives.

v7 design notes (bf16 datapath of v5 + restructured schedule):
  - fp8/DoubleRow was tried (v6) and REVERTED: DR halves matmul row time
    in theory but every fp8 elementwise op (exp, copies, broadcasts,
    reduces) runs 1.5-2x slower on Scalar/DVE/GpSimd, and those engines
    are the real bottleneck; net was a big regression.
  - Startup: gathers ordered k0,k1,q0,q1,k2,k3; XBAR transposes split
    across BOTH hwdge queues (k0,k1,q0 on Sync; q1,k2,k3 on Scalar).
    Phase C (hq projection) runs before D2-pair0 on the PE; only a
    short warmup burst remains for the HAM clock ramp.
  - hqw produced per (m, sample-pair): pairs 0,1 on DVE right after
    phase C (ready before D5 sample 0); pairs 2,3 on GpSimd emitted
    early in the loop. Removes the v5 phase-C stall region entirely.
  - lg PSUM double-buffered (D5 of b+1 no longer waits D6 exp of b);
    mm pool at 2 bufs to pay for it.
  - poT scale on GpSimd (SBUF-only op; DVE is the loop's 2nd engine).
  - Tail: sim-phase Exp uses accum_out (Z partial sums ride the Scalar
    op; no DVE reduces); final (sim - lnZ) split DVE/Scalar with
    staggered output DMAs.
"""

import sys

if "/opt/trn_rl_repo" not in sys.path:
    sys.path.insert(0, "/opt/trn_rl_repo")

import numpy as np

import concourse.bass as bass
import concourse.mybir as mybir
import concourse.tile as tile
from concourse import bacc
from concourse.bass_utils import run_bass_kernel_spmd

F32 = mybir.dt.float32
BF16 = mybir.dt.bfloat16
I32 = mybir.dt.int32
AX = mybir.AxisListType
OP = mybir.AluOpType
AF = mybir.ActivationFunctionType

N_CORES = 8
VOCAB = 20000
E = 300          # word embedding size
EA = 384         # padded: col 300 = ones (bias trick), 301.. = zeros
H = 1024         # hidden
G = 8            # heads
N_OUT = 300
N_ANS = 4000
B, NQ, NK = 64, 32, 256
BL = B // N_CORES            # 8 samples per core
TQ = BL * NQ                 # 256 q tokens per core
TK = BL * NK                 # 2048 k tokens per core
TQ_TILES = TQ // 128         # 2
TK_TILES = TK // 128         # 16
DT = H // 128                # 8 d-tiles
N_CHUNKS = (128, 128, N_OUT - 256)   # (128, 128, 44) rows of the 300-dim
NA_CH = 8                    # sim computed in 8 chunks of 500
NA_W = N_ANS // NA_CH        # 500
NWOUT = G * DT               # 64 Wout k-tiles

DEBUG_TAPS = False


def build_kernel(num_devices=N_CORES):
    nc = bacc.Bacc("TRN2", target_bir_lowering=False, debug=False,
                   num_devices=num_devices)

    # ---- DRAM I/O ----
    emb_d = nc.dram_tensor("emb", [VOCAB, EA], BF16, kind="ExternalInput").ap()
    idxq_d = nc.dram_tensor("idx_q", [128, TQ_TILES], I32, kind="ExternalInput").ap()
    idxk_d = nc.dram_tensor("idx_k", [128, TK_TILES], I32, kind="ExternalInput").ap()
    wq_d = nc.dram_tensor("wq", [EA, H], BF16, kind="ExternalInput").ap()
    wk_d = nc.dram_tensor("wk", [EA, H], BF16, kind="ExternalInput").ap()
    watt_d = nc.dram_tensor("watt", [128, DT, G], BF16, kind="ExternalInput").ap()
    wout_d = nc.dram_tensor("wout", [G * H, N_OUT], BF16, kind="ExternalInput").ap()
    bout_d = nc.dram_tensor("bout", [BL, N_OUT], F32, kind="ExternalInput").ap()
    glovet_d = nc.dram_tensor("glovet", [N_OUT, N_ANS], BF16,
                              kind="ExternalInput").ap()
    out_d = nc.dram_tensor("out", [BL, N_ANS], F32, kind="ExternalOutput").ap()
    warm_d = nc.dram_tensor("warm", [1, 128], F32, kind="ExternalOutput").ap()
    if DEBUG_TAPS:
        dbg_hqt_d = nc.dram_tensor("dbg_hqt", [128, DT, TQ], BF16,
                                   kind="ExternalOutput").ap()
        dbg_hqw_d = nc.dram_tensor("dbg_hqw", [128, DT, 2, G, NQ], BF16,
                                   kind="ExternalOutput").ap()
        dbg_hkt_d = nc.dram_tensor("dbg_hkt", [128, DT, 512], BF16,
                                   kind="ExternalOutput").ap()
        dbg_hk_d = nc.dram_tensor("dbg_hk", [128, 4, H], BF16,
                                  kind="ExternalOutput").ap()
        dbg_et_d = nc.dram_tensor("dbg_et", [128, 2, G * NQ], BF16,
                                  kind="ExternalOutput").ap()
        dbg_vr_d = nc.dram_tensor("dbg_vr", [128, DT, G], F32,
                                  kind="ExternalOutput").ap()
        dbg_pot_d = nc.dram_tensor("dbg_pot", [128, DT, G, BL], BF16,
                                   kind="ExternalOutput").ap()

    with tile.TileContext(nc) as tc:
        import contextlib

        with contextlib.ExitStack() as ctx:
            consts = ctx.enter_context(tc.tile_pool(name="consts", bufs=1))
            actx = contextlib.ExitStack()
            hqw_pool = actx.enter_context(tc.tile_pool(name="hqwp", bufs=1))
            xrow_p = actx.enter_context(tc.tile_pool(name="xrow", bufs=6))
            xkt_p = actx.enter_context(tc.tile_pool(name="xkt", bufs=2))
            hkt_p = actx.enter_context(tc.tile_pool(name="hkt", bufs=2))
            hk_p = actx.enter_context(tc.tile_pool(name="hk", bufs=2))
            et_p = actx.enter_context(tc.tile_pool(name="et", bufs=2))
            v_p = actx.enter_context(tc.tile_pool(name="v", bufs=3))
            vr_p = actx.enter_context(tc.tile_pool(name="vr", bufs=3))
            zz_p = actx.enter_context(tc.tile_pool(name="zz", bufs=2))
            zn_p = actx.enter_context(tc.tile_pool(name="zn", bufs=3))
            mm_p = actx.enter_context(tc.tile_pool(name="mm", bufs=2, space="PSUM"))
            tp_p = actx.enter_context(tc.tile_pool(name="tp", bufs=2, space="PSUM"))
            lg_p = actx.enter_context(tc.tile_pool(name="lg", bufs=2, space="PSUM"))
            up_p = actx.enter_context(tc.tile_pool(name="up", bufs=2, space="PSUM"))

            # ---- on-chip constants (no DMA round-trip) ----
            ident = consts.tile([128, 128], BF16, tag="ident")
            nc.gpsimd.memset(ident[:], 1.0)
            nc.gpsimd.affine_select(
                out=ident[:], in_=ident[:], pattern=[[-1, 128]], base=0,
                channel_multiplier=1, compare_op=OP.is_equal, fill=0.0,
            )
            ones_sb = consts.tile([128, 1], F32, tag="ones")
            nc.gpsimd.memset(ones_sb[:], 1.0)
            ones_row = consts.tile([1, 128], F32, tag="ones_row")
            nc.gpsimd.memset(ones_row[:], 1.0)
            wz = consts.tile([128, 512], BF16, tag="wz")
            nc.vector.memset(wz[:], 0.0)

            # ---- critical input DMAs (sync queue, tiny idx first) ----
            idxk_sb = consts.tile([128, TK_TILES], I32, tag="idxk")
            nc.sync.dma_start(idxk_sb[:], idxk_d)
            idxq_sb = consts.tile([128, TQ_TILES], I32, tag="idxq")
            nc.sync.dma_start(idxq_sb[:], idxq_d)
            wq_sb = consts.tile([128, 3, H], BF16, tag="wq")
            nc.sync.dma_start(
                wq_sb[:], wq_d.rearrange("(c p) h -> p c h", p=128))
            wk_sb = consts.tile([128, 3, H], BF16, tag="wk")
            watt_sb = consts.tile([128, DT, G], BF16, tag="watt")

            # ---- gathers on gpsimd: k0,k1 / q0,q1 / k2,k3 ----
            xrow_tiles = {}

            def gather(idx_sb, col):
                xrow = xrow_p.tile([128, EA], BF16, tag="xrow")
                nc.gpsimd.indirect_dma_start(
                    out=xrow[:],
                    out_offset=None,
                    in_=emb_d,
                    in_offset=bass.IndirectOffsetOnAxis(
                        ap=idx_sb[:, col : col + 1], axis=0
                    ),
                )
                return xrow

            def transpose_x(xrow, dst, dst_col):
                """Transpose one gathered X tile [k, e] -> [e, k] on the PE
                (3 128x128 blocks into PSUM) + one Scalar copy. The XBAR
                DMA_TRANSPOSE path is avoided entirely: hwdge transposes
                serialize ~1.3us each AND acquire conservative waits on ALL
                in-flight DMAs, which repeatedly stalled the critical path
                for tens of us."""
                pt = tp_p.tile([128, 512], BF16, tag="mmT")
                for c in range(3):
                    nc.tensor.transpose(
                        pt[:, c * 128 : (c + 1) * 128],
                        xrow[:, c * 128 : (c + 1) * 128],
                        ident[:],
                    )
                nc.scalar.activation(
                    out=dst[:, :, dst_col * 128 : (dst_col + 1) * 128],
                    in_=pt[:, :384].rearrange("p (c f) -> p c f", c=3),
                    func=AF.Copy)

            # PE warmup: short bf16 burst to push the HAM clock ramp while
            # the gathers land.
            wps = up_p.tile([128, 512], F32, tag="up")
            for _ in range(10):
                nc.tensor.matmul(wps[:], lhsT=ident[:], rhs=wz[:],
                                 start=True, stop=True)
            warm_sb = consts.tile([1, 128], F32, tag="warm")
            nc.vector.tensor_copy(warm_sb[:], wps[:1, :128])
            nc.sync.dma_start(warm_d, warm_sb[:])

            xqT = consts.tile([128, 3, TQ], BF16, tag="xqT")
            xkT0 = xkt_p.tile([128, 3, 512], BF16, tag="xkT")
            for t in range(TQ_TILES):
                xrow_tiles[("q", t)] = gather(idxq_sb, t)
            for t in range(4):
                xrow_tiles[("k", t)] = gather(idxk_sb, t)
            # wk/watt DMAs emitted AFTER the gathers: the PE's first x
            # transpose conservatively waits all DMAs emitted before it
            nc.sync.dma_start(
                wk_sb[:], wk_d.rearrange("(c p) h -> p c h", p=128))
            nc.sync.dma_start(watt_sb[:], watt_d)
            transpose_x(xrow_tiles.pop(("q", 0)), xqT, 0)
            transpose_x(xrow_tiles.pop(("q", 1)), xqT, 1)

            # ---- phase C: hqT (PE before D2 pair0; needs only q gathers) --
            # hqw: ONE TILE PER SAMPLE-PAIR, layout [d, m, 2, g, q]. The
            # single-tile variant with [:, m, 2bp:2bp+2] write slices raced
            # its readers (subtile dep tracking missed the b-sliced 5-D
            # writes); whole-[:, m] writes per tile are tracked correctly.
            hqT = consts.tile([128, DT, TQ], BF16, tag="hqT")
            hqw_t = [hqw_pool.tile([128, DT, 2, G, NQ], BF16, tag=f"hqw{bp}",
                                   name=f"hqw{bp}")
                     for bp in range(4)]
            for m in range(DT):
                ps = mm_p.tile([128, 512], F32, tag="mm")
                for c in range(3):
                    nc.tensor.matmul(
                        ps[:, :TQ],
                        lhsT=wq_sb[:, c, m * 128 : (m + 1) * 128],
                        rhs=xqT[:, c, :],
                        start=(c == 0),
                        stop=(c == 2),
                    )
                nc.vector.tensor_copy(out=hqT[:, m, :], in_=ps[:, :TQ])
            # k-tile transposes on the PE right after phase C (each waits
            # only its own gather's completion)
            for t in range(4):
                transpose_x(xrow_tiles.pop(("k", t)), xkT0, t)

            # ALL hqw ops stay on DVE: offloading them to GpSimd was tried
            # twice and reverted -- concurrent GpSimd+DVE broadcast ops
            # lockstep each other at ~2x duration (shared SBUF path), even
            # with fully private input tensors.
            def emit_hqw(eng, m, bp):
                """hqw for d-tile m, sample pair bp (2 samples)."""
                with nc.allow_low_precision(reason="bf16 hqw"):
                    eng.tensor_tensor(
                        out=hqw_t[bp][:, m],
                        in0=hqT[:, m, :].rearrange("p (b q) -> p b q", b=BL)[
                            :, 2 * bp : 2 * bp + 2, None, :
                        ].to_broadcast([128, 2, G, NQ]),
                        in1=watt_sb[:, m, None, :, None].to_broadcast(
                            [128, 2, G, NQ]),
                        op=OP.mult,
                    )

            # pair 0 on DVE right after the hqT copies: ready before D5
            # sample 0 consumes it (pair 1 emitted after pair0's hkT copies)
            for m in range(DT):
                emit_hqw(nc.vector, m, 0)

            if DEBUG_TAPS:
                nc.sync.dma_start(dbg_hqt_d, hqT[:])
                dbg_hqw = consts.tile([128, DT, 2, G, NQ], BF16, tag="dbg_hqw")
                nc.vector.tensor_copy(dbg_hqw[:], hqw_t[0][:])
                nc.sync.dma_start(dbg_hqw_d, dbg_hqw[:])

            def project_k_pair(xkT):
                """hkT [d, k-pair] by matmul (so D5 never waits on a DMA
                transpose), then token-major hk via PE transposes, lagged
                one d-tile so each hkT copy has landed before its
                transpose reads it."""
                hkT = hkt_p.tile([128, DT, 512], BF16, tag="hkT")
                hk = hk_p.tile([128, 4, H], BF16, tag="hk")

                def emit_d2(m):
                    ps = mm_p.tile([128, 512], F32, tag="mm")
                    for c in range(3):
                        nc.tensor.matmul(
                            ps[:],
                            lhsT=wk_sb[:, c, m * 128 : (m + 1) * 128],
                            rhs=xkT[:, c, :],
                            start=(c == 0),
                            stop=(c == 2),
                        )
                    if m % 2 == 0:
                        nc.scalar.activation(out=hkT[:, m, :], in_=ps[:],
                                             func=AF.Copy)
                    else:
                        nc.vector.tensor_copy(out=hkT[:, m, :], in_=ps[:])

                def emit_transpose(m):
                    pt = tp_p.tile([128, 512], BF16, tag="mmT")
                    for t in range(4):
                        nc.tensor.transpose(
                            pt[:, t * 128 : (t + 1) * 128],
                            hkT[:, m, t * 128 : (t + 1) * 128],
                            ident[:],
                        )
                    nc.scalar.activation(
                        out=hk[:, :, m * 128 : (m + 1) * 128],
                        in_=pt[:].rearrange("p (t f) -> p t f", t=4),
                        func=AF.Copy)

                emit_d2(0)
                for m in range(1, DT):
                    emit_d2(m)
                    emit_transpose(m - 1)
                emit_transpose(DT - 1)
                return hk, hkT

            # ---- D2 pair 0 ----
            hk_cur, hkT_cur = project_k_pair(xkT0)
            # pair 1's hqw after pair0's hkT copies on the DVE queue, so
            # D5 sample 0 isn't gated by these (deadline: sample 2)
            for m in range(DT):
                emit_hqw(nc.vector, m, 1)

            if DEBUG_TAPS:
                dbg_hkt = consts.tile([128, DT, 512], BF16, tag="dbg_hkt")
                nc.vector.tensor_copy(dbg_hkt[:], hkT_cur[:])
                nc.sync.dma_start(dbg_hkt_d, dbg_hkt[:])
                dbg_hk = consts.tile([128, 4, H], BF16, tag="dbg_hk")
                nc.vector.tensor_copy(dbg_hk[:], hk_cur[:])
                nc.sync.dma_start(dbg_hk_d, dbg_hk[:])

            # deferred weight streams, chunked so no single transfer
            # monopolizes the DMA path or skews semaphore completion order
            wout_sb = consts.tile([128, NWOUT, N_OUT], BF16, tag="wout")
            glove_sb = consts.tile([128, 3, N_ANS], BF16, tag="glove")
            bout_sb = consts.tile([BL, N_OUT], F32, tag="bout")

            def emit_weight_chunk(step):
                if step < 8:        # wout: 8 chunks of 8 k-tiles
                    lo = step * 8
                    nc.sync.dma_start(
                        wout_sb[:, lo : lo + 8, :],
                        wout_d[lo * 128 : (lo + 8) * 128].rearrange(
                            "(t p) n -> p t n", p=128))
                elif step < 10:     # glove rows 0..255 in 2 chunks
                    c = step - 8
                    nc.sync.dma_start(
                        glove_sb[:, c, :],
                        glovet_d[c * 128 : (c + 1) * 128])
                elif step == 10:    # glove rows 256..299
                    nc.sync.dma_start(glove_sb[: N_OUT - 256, 2, :],
                                      glovet_d[2 * 128 : N_OUT])
                elif step == 11:
                    nc.sync.dma_start(bout_sb[:], bout_d)

            poT = consts.tile([128, DT, G, BL], BF16, tag="poT")
            # unscaled pooled accumulator + per-sample 1/Z values: the
            # pooled scale is applied ONCE after the loop. The per-sample
            # recip->partition_broadcast->scale chain (DVE->GpSimd->DVE)
            # paid ~2.3us of semaphore latency per hop and head-of-line
            # blocked the in-order DVE queue, stalling the next sample.
            vrT = consts.tile([128, DT, G, BL], F32, tag="vrT")
            zall = consts.tile([1, G, BL], F32, tag="zall")
            wstep = 0

            # ---- phase D: attention, two samples per pair ----
            for p in range(BL // 2):
                hk, hkT = hk_cur, hkT_cur
                xkT_next = None
                if p < 3:
                    xkT_next = xkt_p.tile([128, 3, 512], BF16, tag="xkT")
                    for t in range(4):
                        xrow_tiles[("k", t)] = gather(idxk_sb, (p + 1) * 4 + t)


                for bi in range(2):
                    b = p * 2 + bi

                    # D5: logits.T [k, (g,q)] in PSUM: [128, 2, 256]
                    ps_l = lg_p.tile([128, 2, 256], F32, tag="lg")
                    for kt in range(2):
                        for c in range(DT):
                            nc.tensor.matmul(
                                ps_l[:, kt, :],
                                lhsT=hkT[
                                    :, c,
                                    bi * 256 + kt * 128 : bi * 256 + (kt + 1) * 128,
                                ],
                                rhs=hqw_t[b // 2][:, c, b % 2],
                                start=(c == 0),
                                stop=(c == DT - 1),
                            )

                    # D6: E = exp(logits) bf16 (one op), zz sums (one op)
                    et = et_p.tile([128, 2, G * NQ], BF16, tag="et")
                    zz = zz_p.tile([128, 2, G], F32, tag="zz")
                    nc.scalar.activation(
                        out=et[:], in_=ps_l[:], func=AF.Exp)
                    nc.vector.tensor_reduce(
                        out=zz[:],
                        in_=et[:].rearrange("p t (g q) -> p t g q", g=G),
                        axis=AX.X,
                        op=OP.add,
                    )

                    def emit_d7():
                        # D7: Z_g over k-partitions, 1/Z stashed for the
                        # batched scale. Emitted late (post-D8) so the PE
                        # never waits on zz -- except for the LAST sample,
                        # where it runs early so recip lands before the
                        # final pooled scale that gates phase F.
                        ps_z = mm_p.tile([128, 512], F32, tag="mm")
                        for kt in range(2):
                            nc.tensor.matmul(
                                ps_z[:1, :G],
                                lhsT=ones_sb[:],
                                rhs=zz[:, kt, :],
                                start=(kt == 0),
                                stop=(kt == 1),
                            )
                        nc.vector.reciprocal(zall[:1, :, b], ps_z[:1, :G])

                    if b == BL - 1:
                        emit_d7()

                    # D8: u = hk.T @ E per 2 d-tiles; v = u * hq; vr = sum_q
                    for mp in range(4):
                        ps_u = up_p.tile([128, 512], F32, tag="up")
                        for mi in range(2):
                            m = mp * 2 + mi
                            for kt in range(2):
                                nc.tensor.matmul(
                                    ps_u[:, mi * 256 : (mi + 1) * 256],
                                    lhsT=hk[:, bi * 2 + kt, m * 128 : (m + 1) * 128],
                                    rhs=et[:, kt, :],
                                    start=(kt == 0),
                                    stop=(kt == 1),
                                )
                        v = v_p.tile([128, 2, G, NQ], BF16, tag="v")
                        with nc.allow_low_precision(reason="bf16 v"):
                            nc.vector.tensor_tensor(
                                out=v[:],
                                in0=ps_u[:].rearrange(
                                    "p (m g q) -> p m g q", m=2, g=G),
                                in1=hqT[
                                    :, mp * 2 : mp * 2 + 2, None,
                                    b * NQ : (b + 1) * NQ
                                ].to_broadcast([128, 2, G, NQ]),
                                op=OP.mult,
                            )
                        nc.vector.tensor_reduce(
                            out=vrT[:, mp * 2 : mp * 2 + 2, :, b], in_=v[:],
                            axis=AX.X, op=OP.add,
                        )

                    if DEBUG_TAPS and b == 0:
                        dbg_et = consts.tile([128, 2, G * NQ], BF16,
                                             tag="dbg_et")
                        nc.vector.tensor_copy(dbg_et[:], et[:])
                        nc.sync.dma_start(dbg_et_d, dbg_et[:])
                        nc.sync.dma_start(dbg_vr_d, vrT[:, :, :, 0])

                    if b != BL - 1:
                        emit_d7()
                    # hqw for pairs 2,3 in 4-op slices woven into the DVE
                    # queue across samples 0-3 (fine grain so the up-pool
                    # rotation never backs up the PE)
                    if b < 4:
                        for mi in range(4):
                            emit_hqw(nc.vector, (b % 2) * 4 + mi, 2 + b // 2)

                    if bi == 0:
                        if p < 3:
                            # next pair's X transposes mid-pair (PE + Scalar
                            # copies; each waits only its own gather)
                            for t in range(4):
                                transpose_x(xrow_tiles.pop(("k", t)),
                                            xkT_next, t)
                        emit_weight_chunk(wstep); wstep += 1
                        emit_weight_chunk(wstep); wstep += 1

                emit_weight_chunk(wstep); wstep += 1
                if p < 3:
                    hk_cur, hkT_cur = project_k_pair(xkT_next)

            # batched pooled scale: one PE broadcast of 1/Z to all
            # partitions, one DVE multiply for all 8 samples
            zb_ps = mm_p.tile([128, 512], F32, tag="mm")
            nc.tensor.matmul(
                zb_ps[:, : G * BL],
                lhsT=ones_row[:],
                rhs=zall[:1].rearrange("o g b -> o (g b)"),
                start=True, stop=True,
            )
            with nc.allow_low_precision(reason="bf16 pooled"):
                nc.vector.tensor_tensor(
                    out=poT[:],
                    in0=vrT[:],
                    in1=zb_ps[:, : G * BL].rearrange(
                        "p (g b) -> p g b", g=G)[:, None, :, :]
                    .to_broadcast([128, DT, G, BL]),
                    op=OP.mult,
                )

            if DEBUG_TAPS:
                nc.sync.dma_start(dbg_pot_d, poT[:])

            # attention pools (incl. all PSUM) are dead now
            actx.close()
            fctx = contextlib.ExitStack()
            fo_p = fctx.enter_context(tc.tile_pool(name="fo", bufs=2, space="PSUM"))

            # ---- phase F: out [8, 300] = pooled_flat @ Wout + bout ----
            ps_o = fo_p.tile([128, 512], F32, tag="fo")
            for g in range(G):
                for m in range(DT):
                    t = g * DT + m
                    nc.tensor.matmul(
                        ps_o[:BL, :N_OUT],
                        lhsT=poT[:, m, g, :],
                        rhs=wout_sb[:, t, :],
                        start=(t == 0),
                        stop=(t == NWOUT - 1),
                    )
            out_sb = consts.tile([BL, N_OUT], BF16, tag="out_sb")
            with nc.allow_low_precision(reason="bf16 out"):
                nc.vector.tensor_tensor(
                    out=out_sb[:], in0=ps_o[:BL, :N_OUT], in1=bout_sb[:], op=OP.add
                )

            # ---- phase G: sim + log_softmax (no max shift; sim is O(+-5)) --
            outT = consts.tile([128, 3, BL], BF16, tag="outT")
            for c, rows in enumerate(N_CHUNKS):
                psT = fo_p.tile([128, 128], BF16, tag="foT")
                nc.tensor.transpose(
                    psT[:rows, :BL],
                    out_sb[:, c * 128 : c * 128 + rows],
                    ident[:BL, :BL],
                )
                nc.scalar.activation(out=outT[:rows, c, :], in_=psT[:rows, :BL],
                                     func=AF.Copy)

            zs8 = consts.tile([BL, NA_CH], F32, tag="zs8")
            zs = consts.tile([BL, 1], F32, tag="zs")
            zsi = consts.tile([BL, 1], F32, tag="zsi")
            nlnz = consts.tile([BL, 1], F32, tag="nlnz")
            final_sb = consts.tile([BL, N_ANS], F32, tag="final")
            simsb = consts.tile([BL, N_ANS], BF16, tag="simsb")

            # sim chunks on 4 rotating PSUM banks alongside fo's 2: no
            # fctx.close() barrier (its PE DRAIN burned ~7us mid-tail).
            # Each chunk is staged to SBUF bf16 so its bank recycles; the
            # final subtract reads the staged copy.
            esc_p = fctx.enter_context(tc.tile_pool(name="esc", bufs=2))
            sim_p = fctx.enter_context(tc.tile_pool(name="simp", bufs=4,
                                                    space="PSUM"))
            for a in range(NA_CH):
                span = slice(a * NA_W, (a + 1) * NA_W)
                ps_s = sim_p.tile([128, NA_W], F32, tag="simp")
                for c, rows in enumerate(N_CHUNKS):
                    nc.tensor.matmul(
                        ps_s[:BL, :],
                        lhsT=outT[:rows, c, :],
                        rhs=glove_sb[:rows, c, a * NA_W : (a + 1) * NA_W],
                        start=(c == 0),
                        stop=(c == 2),
                    )
                # Exp + DVE reduce (accum_out was tried: the per-op
                # ACTIVATION_READ_ACCUMULATOR + semaphore round-trip
                # serialized the Scalar chain at 1.43us/chunk)
                esc = esc_p.tile([BL, NA_W], BF16, tag="esc")
                nc.scalar.activation(out=esc[:], in_=ps_s[:BL, :], func=AF.Exp)
                nc.vector.tensor_reduce(
                    out=zs8[:, a : a + 1], in_=esc[:], axis=AX.X, op=OP.add
                )
                with nc.allow_low_precision(reason="bf16 sim stage"):
                    if a % 2 == 0:
                        nc.vector.tensor_copy(out=simsb[:, span],
                                              in_=ps_s[:BL, :])
                    else:
                        nc.scalar.activation(out=simsb[:, span],
                                             in_=ps_s[:BL, :], func=AF.Copy)

            nc.vector.tensor_reduce(out=zs[:], in_=zs8[:], axis=AX.X, op=OP.add)
            nc.vector.reciprocal(zsi[:], zs[:])
            nc.scalar.activation(out=nlnz[:], in_=zsi[:], func=AF.Ln)
            # final = sim - lnZ, chunks split DVE / Act (GpSimd was tried:
            # on 8-partition tiles its ops run 5-8us each), staggered DMAs
            for a in range(NA_CH):
                span = slice(a * NA_W, (a + 1) * NA_W)
                if a % 2 == 0:
                    nc.vector.tensor_scalar(
                        out=final_sb[:, span], in0=simsb[:, span],
                        scalar1=nlnz[:], scalar2=None,
                        op0=OP.add,
                    )
                else:
                    nc.scalar.activation(
                        out=final_sb[:, span], in_=simsb[:, span],
                        func=AF.Identity, bias=nlnz[:],
                    )
                if a == 2:
                    nc.sync.dma_start(out_d[:, : 3 * NA_W],
                                      final_sb[:, : 3 * NA_W])
                if a == 5:
                    nc.sync.dma_start(out_d[:, 3 * NA_W : 6 * NA_W],
                                      final_sb[:, 3 * NA_W : 6 * NA_W])
            nc.sync.dma_start(out_d[:, 6 * NA_W :], final_sb[:, 6 * NA_W :])
            fctx.close()

    nc.compile()
    return nc


_NC = None


def _get_nc():
    global _NC
    if _NC is None:
        _NC = build_kernel()
    return _NC


def make_in_maps(inputs):
    import ml_dtypes

    bf = ml_dtypes.bfloat16
    he_q = np.asarray(inputs["he_ques"]).astype(np.int32)   # [64, 32]
    he_k = np.asarray(inputs["he_kg"]).astype(np.int32)     # [64, 256]
    emb0 = np.asarray(inputs["emb"], dtype=np.float32)
    emb = np.zeros((VOCAB, EA), dtype=bf)
    emb[:, :E] = emb0.astype(bf)
    emb[:, E] = np.ones((), dtype=bf)                       # bias column
    wq = np.zeros((EA, H), dtype=bf)
    wq[:E] = np.asarray(inputs["Wq"], np.float32).astype(bf)
    wq[E] = np.asarray(inputs["bq"], np.float32).astype(bf)
    wk = np.zeros((EA, H), dtype=bf)
    wk[:E] = np.asarray(inputs["Wk"], np.float32).astype(bf)
    wk[E] = np.asarray(inputs["bk"], np.float32).astype(bf)
    watt = np.ascontiguousarray(
        np.asarray(inputs["Watt"], np.float32).reshape(DT, 128, G)
        .transpose(1, 0, 2)).astype(bf)                     # [128, DT, G]
    wout = np.ascontiguousarray(
        np.asarray(inputs["Wout"], np.float32)).astype(bf)
    bout = np.ascontiguousarray(
        np.broadcast_to(np.asarray(inputs["bout"], np.float32), (BL, N_OUT)))
    glovet = np.ascontiguousarray(
        np.asarray(inputs["glove_cands"], np.float32).T).astype(bf)  # [300,4000]

    in_maps = []
    for i in range(N_CORES):
        iq = he_q[i * BL : (i + 1) * BL].reshape(-1)        # [256]
        ik = he_k[i * BL : (i + 1) * BL].reshape(-1)        # [2048]
        in_maps.append({
            "emb": emb,
            "idx_q": np.ascontiguousarray(iq.reshape(TQ_TILES, 128).T),
            "idx_k": np.ascontiguousarray(ik.reshape(TK_TILES, 128).T),
            "wq": wq,
            "wk": wk,
            "watt": watt,
            "wout": wout,
            "bout": bout,
            "glovet": glovet,
        })
    return in_maps


def kernel(**inputs) -> np.ndarray:
    nc = _get_nc()
    in_maps = make_in_maps(inputs)
    res = run_bass_kernel_spmd(nc, in_maps, list(range(N_CORES)))
    return np.concatenate(
        [np.asarray(res.results[i]["out"], np.float32) for i in range(N_CORES)],
        axis=0,
    )
